# revision 9
# baseline (speedup 1.0000x reference)
"""SMPL forward (shape blendshapes + pose blendshapes + FK + LBS) on 8 TRN2 NeuronCores.

Data-parallel: batch 1024 -> 128 per core. Layout: batch on the 128 SBUF
partitions; vertex streams along the free dimension, y/x-planar, V padded
7168 = 14 tiles of 512.

Pipeline per core:
  stage A (f32, small): Rodrigues rotations, joints from shape (host-folded
    J_regressor algebra), forward kinematics over the joint tree, A_rel.
  stage B (bf16 matmuls + elementwise): v_posed via one accumulated matmul
    [beta,1,pose_feat] @ [shapedirs;v_template;posedirs]; per-vertex blended
    transforms Tv_c = A_rel_c^T @ lbs_weights^T (12 components, 4-way
    row-packed K=24 matmuls); apply verts = Tv[:, :3,:3] p + Tv[:, :3,3].
"""

import sys

for _p in ("/opt/trn_rl_repo",):
    if _p not in sys.path:
        sys.path.append(_p)

import numpy as np
import ml_dtypes

import concourse.bass as bass
import concourse.mybir as mybir
from concourse.tile import TileContext
from concourse.vector_clock import ScopedClock
from concourse.masks import make_identity
from concourse import bass_utils

BF16 = ml_dtypes.bfloat16

B, V, J, NB = 1024, 6890, 24, 10
NCORES = 8
BL = B // NCORES            # 128 batches per core
VP = 7168                   # padded vertex count (14 * 512)
VT = 512                    # vertex tile
NT = VP // VT               # 14
KP = NB + 1 + (J - 1) * 9   # 218 = [beta(10), 1, pose_feat(207)]

PARENTS = np.array(
    [-1, 0, 0, 0, 1, 2, 3, 4, 5, 6, 7, 8, 9, 9, 9, 12, 13, 14, 16, 17, 18, 19, 20, 21],
    dtype=np.int32,
)

# FK levels: (joint_start, count, parent_start, parent_step) — parents are
# either contiguous (step 1 in joint index) or a single repeated joint (step 0).
FK_LEVELS = [
    (1, 3, 0, 0),
    (4, 3, 1, 1),
    (7, 3, 4, 1),
    (10, 3, 7, 1),
    (13, 2, 9, 0),
    (15, 3, 12, 1),
    (18, 2, 16, 1),
    (20, 2, 18, 1),
    (22, 2, 20, 1),
]

F32 = mybir.dt.float32
BF = mybir.dt.bfloat16


def _patched_drain_and_barrier(self, tick_clock, wait_clock):
    # This container's walrus accepts at most ONE sync-wait on a CTRL/Drain
    # instruction. Spread the end-of-kernel waits over SP NOPs, one each.
    nc = self.nc
    placeholders = [nc.sync.nop(nofuse=True) for _ in range(31)]
    drain_inst = nc.sync.drain()
    wait_clock.add_sem_waits(
        drain_inst.ins, ScopedClock({None: tick_clock.global_clock})
    )
    si = drain_inst.ins.sync_info
    waits = list(si.on_wait) if si and si.on_wait else []
    if len(waits) > 1:
        assert len(waits) - 1 <= len(placeholders), f"{len(waits)} drain waits"
        for ph, w in zip(placeholders, waits[:-1]):
            ph.ins.sync_info = mybir.SyncInfo(on_wait=[w], on_update=[])
        upd = list(si.on_update) if si.on_update else []
        drain_inst.ins.sync_info = mybir.SyncInfo(on_wait=[waits[-1]], on_update=upd)

    nc.all_engine_barrier()
    assert self.sems is not None
    popped = nc._tile_sem_poison_stack.pop()
    assert popped is self._sem_poison
    nc.clear_and_free_semaphores(list(self.sems.allocated().values()))
    nc.all_engine_barrier()


TileContext._drain_and_barrier = _patched_drain_and_barrier


def _split_multi_waits(nc):
    """This container's walrus accepts at most one sync-wait per instruction.
    Move surplus waits onto same-engine NOPs inserted just before."""
    uid = 0
    for fn in nc.m.functions:
        for bb in fn.blocks:
            changed = False
            new = []
            for inst in bb.instructions:
                si = inst.sync_info
                waits = list(si.on_wait) if si and si.on_wait else []
                if len(waits) > 1:
                    changed = True
                    for w in waits[:-1]:
                        uid += 1
                        nop = mybir.InstNoOp(
                            name=f"I-waitsplit-{uid}", ins=[], outs=[]
                        )
                        nop.engine = inst.engine
                        nop.sync_info = mybir.SyncInfo(on_wait=[w], on_update=[])
                        new.append(nop)
                    upd = list(si.on_update) if si.on_update else []
                    inst.sync_info = mybir.SyncInfo(
                        on_wait=[waits[-1]], on_update=upd
                    )
                new.append(inst)
            if changed:
                bb.instructions = new
    return nc


# --------------------------------------------------------------------------
# Host-side constant folding (model buffers only — no batch inputs touched)
# --------------------------------------------------------------------------

def _prep_consts(v_template, shapedirs, posedirs, lbs_weights, J_regressor):
    # dirs (218, 3*VP), y-planar columns: col = y*VP + v
    dirs = np.zeros((KP, 3 * VP), np.float32)
    dirs[0:NB].reshape(NB, 3, VP)[:, :, :V] = shapedirs.transpose(2, 1, 0)
    dirs[NB].reshape(3, VP)[:, :V] = v_template.T
    dirs[NB + 1:].reshape(207, 3, VP)[:, :, :V] = (
        posedirs.reshape(207, V, 3).transpose(0, 2, 1)
    )
    dirsA = np.ascontiguousarray(dirs[:128]).astype(BF16)
    dirsB = np.ascontiguousarray(dirs[128:]).astype(BF16)

    # lbs weights transposed, replicated on 4 partition strips of 32
    w4 = np.zeros((128, VP), np.float32)
    for i in range(4):
        w4[32 * i:32 * i + J, :V] = lbs_weights.T
    w4 = w4.astype(BF16)

    # joints = J_regressor @ v_shaped folded to:  joints = base + jd @ beta
    jd = np.einsum("jv,vcl->jcl", J_regressor, shapedirs)  # (24,3,10)
    bj = J_regressor @ v_template                          # (24,3)
    abs_blk = np.concatenate([jd.reshape(J * 3, NB).T, bj.reshape(1, J * 3)], 0)
    # rel block: rel[j] = abs[j] - abs[parent[j]]
    rel = abs_blk.reshape(NB + 1, J, 3).copy()
    rel[:, 1:, :] -= abs_blk.reshape(NB + 1, J, 3)[:, PARENTS[1:], :]
    rhsj = np.concatenate([rel.reshape(NB + 1, J * 3), abs_blk], axis=1)  # (11,144)
    return dirsA, dirsB, w4, rhsj.astype(np.float32)


# --------------------------------------------------------------------------
# Device kernel
# --------------------------------------------------------------------------

def build_nc():
    nc = bass.Bass()

    shp = nc.dram_tensor("shp", [BL, NB], F32, kind="ExternalInput")
    pose = nc.dram_tensor("pose", [BL, J * 3], F32, kind="ExternalInput")
    gt = nc.dram_tensor("gt", [BL, 3], F32, kind="ExternalInput")
    rhsj = nc.dram_tensor("rhsj", [NB + 1, 144], F32, kind="ExternalInput")
    dirsA = nc.dram_tensor("dirsA", [128, 3 * VP], BF, kind="ExternalInput")
    dirsB = nc.dram_tensor("dirsB", [KP - 128, 3 * VP], BF, kind="ExternalInput")
    w4 = nc.dram_tensor("w4", [128, VP], BF, kind="ExternalInput")
    out = nc.dram_tensor("out", [BL, 3 * VP], F32, kind="ExternalOutput")

    add = mybir.AluOpType.add

    with TileContext(nc) as tc:
        with (
            tc.tile_pool(name="const", bufs=1) as constp,
            tc.tile_pool(name="stagea", bufs=1) as sa,
            tc.tile_pool(name="work", bufs=2) as wk,
            tc.tile_pool(name="tvpool", bufs=2) as tvp,
            tc.tile_pool(name="ppool", bufs=2) as ppl,
            tc.tile_pool(name="outp", bufs=2) as outp,
        ):
            # ---- resident constants ----
            DA = constp.tile([128, 3 * VP], BF)
            DB = constp.tile([KP - 128, 3 * VP], BF)
            W4 = constp.tile([128, VP], BF)
            IDENT = constp.tile([128, 128], F32)
            nc.sync.dma_start(DA[:], dirsA[:])
            nc.sync.dma_start(DB[:], dirsB[:])
            nc.sync.dma_start(W4[:], w4[:])
            make_identity(nc, IDENT[:])

            # ---- stage A inputs ----
            POSE = sa.tile([128, J * 3], F32)
            PB = sa.tile([128, KP], F32)          # [beta | 1 | pose_feat]
            G = sa.tile([128, 3], F32)
            RJ = sa.tile([NB + 1, 144], F32)
            nc.sync.dma_start(POSE[:], pose[:])
            nc.sync.dma_start(PB[:, 0:NB], shp[:])
            nc.sync.dma_start(G[:], gt[:])
            nc.sync.dma_start(RJ[:], rhsj[:])
            nc.vector.memset(PB[:, NB:NB + 1], 1.0)

            # ---- Rodrigues ----
            SQ = sa.tile([128, J * 3], F32)
            TH = sa.tile([128, J], F32)
            SN = sa.tile([128, J], F32)
            CS = sa.tile([128, J], F32)
            OMC = sa.tile([128, J], F32)
            IT = sa.tile([128, J], F32)
            AX = sa.tile([128, J * 3], F32)
            SAT = sa.tile([128, J * 3], F32)
            R = sa.tile([128, J * 9], F32)

            EPS = sa.tile([128, 1], F32)
            HPI = sa.tile([128, 1], F32)
            nc.vector.memset(EPS[:], 1e-8)
            nc.vector.memset(HPI[:], float(np.pi / 2))

            nc.vector.tensor_mul(SQ[:], POSE[:], POSE[:])
            nc.vector.tensor_reduce(
                TH[:], SQ[:].rearrange("p (j y) -> p j y", y=3),
                axis=mybir.AxisListType.X, op=add,
            )
            nc.scalar.activation(TH[:], TH[:], mybir.ActivationFunctionType.Sqrt,
                                 bias=EPS[:])
            nc.scalar.activation(SN[:], TH[:], mybir.ActivationFunctionType.Sin)
            nc.scalar.activation(CS[:], TH[:], mybir.ActivationFunctionType.Sin,
                                 bias=HPI[:])
            nc.vector.reciprocal(IT[:], TH[:])

            p_j3 = lambda t: t[:].rearrange("p (j y) -> p j y", y=3)
            p_j1 = lambda t: t[:].rearrange("p (j o) -> p j o", o=1)
            nc.vector.tensor_mul(
                p_j3(AX), p_j3(POSE), p_j1(IT).broadcast_to([128, J, 3])
            )
            R4 = R[:].rearrange("p (j x y) -> p j x y", x=3, y=3)
            nc.vector.tensor_mul(
                R4,
                p_j3(AX).unsqueeze(3).broadcast_to([128, J, 3, 3]),
                p_j3(AX).unsqueeze(2).broadcast_to([128, J, 3, 3]),
            )
            # (1 - cos)
            nc.scalar.activation(OMC[:], CS[:], mybir.ActivationFunctionType.Copy,
                                 bias=1.0, scale=-1.0)
            nc.vector.tensor_mul(
                R4, R4, p_j1(OMC).unsqueeze(3).broadcast_to([128, J, 3, 3])
            )
            R9 = R[:].rearrange("p (j d) -> p j d", d=9)
            Rdiag = R9[:, :, 0:9:4]
            nc.vector.tensor_add(
                Rdiag, Rdiag, p_j1(CS).broadcast_to([128, J, 3])
            )
            nc.vector.tensor_mul(
                p_j3(SAT), p_j3(AX), p_j1(SN).broadcast_to([128, J, 3])
            )
            SA3 = SAT[:].rearrange("p (j a) -> p j a", a=3)
            # +s*K off-diagonals: (dst_in_9, axis_comp, sign)
            for off, a, sgn in ((1, 2, -1), (2, 1, +1), (3, 2, +1),
                                (5, 0, -1), (6, 1, -1), (7, 0, +1)):
                fn = nc.vector.tensor_add if sgn > 0 else nc.vector.tensor_sub
                fn(R9[:, :, off:off + 1], R9[:, :, off:off + 1], SA3[:, :, a:a + 1])

            # ---- pose_feat into PB ----
            nc.vector.tensor_copy(PB[:, NB + 1:], R[:, 9:])
            PBd = PB[:, NB + 1:].rearrange("p (j d) -> p j d", d=9)[:, :, 0:9:4]
            nc.vector.tensor_scalar_add(PBd, PBd, -1.0)

            # ---- transpose PB -> lhsT chunks; joints matmul ----
            psA_cm = tc.tile_pool(name="psA", bufs=2, space="PSUM")
            psA = psA_cm.__enter__()
            PBT1 = sa.tile([128, 128], BF)
            PBT2 = sa.tile([KP - 128, 128], BF)
            JT = sa.tile([NB + 1, 128], F32)
            t1 = psA.tile([128, 128], F32, tag="ta")
            nc.tensor.transpose(t1[:], PB[:, 0:128], IDENT[:])
            nc.vector.tensor_copy(PBT1[:], t1[:])
            nc.vector.tensor_copy(JT[:], t1[0:NB + 1, :])
            t2 = psA.tile([KP - 128, 128], F32, tag="ta")
            nc.tensor.transpose(t2[:], PB[:, 128:KP], IDENT[:])
            nc.vector.tensor_copy(PBT2[:], t2[:])

            JNT = sa.tile([128, 144], F32)
            jp = psA.tile([128, 144], F32, tag="ta")
            nc.tensor.matmul(jp[:], JT[:], RJ[:], start=True, stop=True)
            nc.vector.tensor_copy(JNT[:], jp[:])

            # ---- FK ----
            TL = sa.tile([128, J * 12], F32)
            TW = sa.tile([128, J * 12], F32)
            TMP = sa.tile([128, 36], F32)
            TLv = TL[:].rearrange("p (j r c) -> p j r c", r=3, c=4)
            TWv = TW[:].rearrange("p (j r c) -> p j r c", r=3, c=4)
            RELv = JNT[:, 0:72].rearrange("p (j k) -> p j k", k=3)
            ABSv = JNT[:, 72:144].rearrange("p (j k) -> p j k", k=3)
            nc.vector.tensor_copy(
                TLv[:, :, :, 0:3], R[:].rearrange("p (j r y) -> p j r y", r=3, y=3)
            )
            nc.vector.tensor_copy(TLv[:, :, :, 3:4], RELv.unsqueeze(3))
            nc.vector.tensor_copy(TWv[:, 0:1, :, :], TLv[:, 0:1, :, :])

            TMPv = TMP[:].rearrange("p (j r c) -> p j r c", r=3, c=4)
            for j0, L, pstart, pstep in FK_LEVELS:
                outv = TWv[:, j0:j0 + L, :, :]
                tmpv = TMPv[:, 0:L, :, :]
                if pstep == 0:
                    parv = lambda k: _bc2(TWv, pstart, L, k)
                else:
                    parv = lambda k: TWv[:, pstart:pstart + L, :, k:k + 1] \
                        .broadcast_to([128, L, 3, 4])
                locv = lambda k: TLv[:, j0:j0 + L, k:k + 1, :] \
                    .broadcast_to([128, L, 3, 4])
                nc.vector.tensor_mul(outv, parv(0), locv(0))
                nc.vector.tensor_mul(tmpv, parv(1), locv(1))
                nc.vector.tensor_add(outv, outv, tmpv)
                nc.vector.tensor_mul(tmpv, parv(2), locv(2))
                nc.vector.tensor_add(outv, outv, tmpv)
                if pstep == 0:
                    ptr = TWv[:, pstart:pstart + 1, :, 3:4].broadcast_to([128, L, 3, 1])
                else:
                    ptr = TWv[:, pstart:pstart + L, :, 3:4]
                nc.vector.tensor_add(
                    TWv[:, j0:j0 + L, :, 3:4], TWv[:, j0:j0 + L, :, 3:4], ptr
                )

            # ---- A_rel translation: tcr = tw - Rw @ j_abs + g ----
            M = sa.tile([128, J * 9], F32)
            TC = sa.tile([128, J * 3], F32)
            TCR = sa.tile([128, J * 3], F32)
            Mv = M[:].rearrange("p (j r k) -> p j r k", r=3, k=3)
            nc.vector.tensor_mul(
                Mv, TWv[:, :, :, 0:3],
                ABSv.unsqueeze(2).broadcast_to([128, J, 3, 3]),
            )
            TCv = TC[:].rearrange("p (j r) -> p j r", r=3)
            nc.vector.tensor_reduce(TCv, Mv, axis=mybir.AxisListType.X, op=add)
            TCRv = TCR[:].rearrange("p (j r) -> p j r", r=3)
            nc.vector.tensor_sub(
                TCRv.unsqueeze(3), TWv[:, :, :, 3:4], TCv.unsqueeze(3)
            )
            nc.vector.tensor_add(
                TCRv.unsqueeze(3), TCRv.unsqueeze(3),
                G[:].rearrange("p (o c) -> p o c", o=1).unsqueeze(3)
                .broadcast_to([128, J, 3, 1]),
            )

            # ---- assemble AR (c-major, 32-padded j) and transpose ----
            AR = sa.tile([128, 384], F32)
            nc.vector.memset(AR[:], 0.0)
            ARv = AR[:].rearrange("p (r co j) -> p r co j", r=3, co=4, j=32)
            nc.vector.tensor_copy(
                ARv[:, :, 0:3, 0:J],
                TWv[:, :, :, 0:3].transpose([0, 2, 3, 1]),
            )
            nc.vector.tensor_copy(
                ARv[:, :, 3:4, 0:J],
                TCRv.transpose([0, 2, 1]).unsqueeze(2),
            )
            ART = []
            for i in range(3):
                ta = psA.tile([128, 128], F32, tag="ta")
                nc.tensor.transpose(ta[:], AR[:, 128 * i:128 * (i + 1)], IDENT[:])
                asb = sa.tile([128, 128], BF, tag=f"art{i}")
                nc.vector.tensor_copy(asb[:], ta[:])
                ART.append(asb)

            psA_cm.__exit__(None, None, None)

            # ---- stage B: per-vertex-tile stream ----
            psP_cm = tc.tile_pool(name="psP", bufs=3, space="PSUM")
            psT_cm = tc.tile_pool(name="psT", bufs=4, space="PSUM")
            psP = psP_cm.__enter__()
            psT = psT_cm.__enter__()
            for t in range(NT):
                # v_posed: psum_y = [beta,1,posefeat] @ dirs   (K = 218)
                pps = []
                for y in range(3):
                    pp = psP.tile([128, VT], F32, tag="pp")
                    c0 = y * VP + t * VT
                    nc.tensor.matmul(pp[:], PBT1[:], DA[:, c0:c0 + VT],
                                     start=True, stop=False)
                    nc.tensor.matmul(pp[:], PBT2[:], DB[:, c0:c0 + VT],
                                     start=False, stop=True)
                    pps.append(pp)
                P = []
                for y in range(3):
                    psb = ppl.tile([128, VT], BF, tag=f"p{y}")
                    nc.scalar.copy(psb[:], pps[y][:])
                    P.append(psb)

                # Tv components: 3 rounds of 4 row-packed K=24 matmuls
                TV = []
                for rnd in range(3):
                    tvps = []
                    for i in range(4):
                        tv = psT.tile([128, VT], F32, tag="tv")
                        nc.tensor.matmul(
                            tv[:],
                            ART[rnd][32 * i:32 * i + J, :],
                            W4[32 * i:32 * i + J, t * VT:(t + 1) * VT],
                            start=True, stop=True,
                            tile_position=(32 * i, 0),
                        )
                        tvps.append(tv)
                    for i in range(4):
                        c = 4 * rnd + i
                        tsb = tvp.tile([128, VT], BF, tag=f"tv{c}")
                        if i % 2 == 0:
                            nc.vector.tensor_copy(tsb[:], tvps[i][:])
                        else:
                            nc.scalar.copy(tsb[:], tvps[i][:])
                        TV.append(tsb)

                # apply
                OUT = outp.tile([128, 3 * VT], F32, tag="out")
                OUTv = OUT[:].rearrange("p (v x) -> p v x", x=3)
                for r in range(3):
                    m0 = wk.tile([128, VT], BF, tag="m0")
                    m1 = wk.tile([128, VT], BF, tag="m1")
                    m2 = wk.tile([128, VT], BF, tag="m2")
                    nc.vector.tensor_mul(m0[:], TV[4 * r + 0][:], P[0][:])
                    nc.vector.tensor_mul(m1[:], TV[4 * r + 1][:], P[1][:])
                    nc.vector.tensor_mul(m2[:], TV[4 * r + 2][:], P[2][:])
                    nc.vector.tensor_add(m0[:], m0[:], m1[:])
                    nc.vector.tensor_add(m2[:], m2[:], TV[4 * r + 3][:])
                    nc.vector.tensor_add(
                        OUTv[:, :, r:r + 1], m0[:].unsqueeze(2), m2[:].unsqueeze(2)
                    )
                nc.sync.dma_start(out[:, t * 3 * VT:(t + 1) * 3 * VT], OUT[:])
            psT_cm.__exit__(None, None, None)
            psP_cm.__exit__(None, None, None)

    return _split_multi_waits(nc)


def _bc2(TWv, pstart, L, k):
    # broadcast a single parent's column k over L joints and 4 columns
    v = TWv[:, pstart:pstart + 1, :, k:k + 1]           # (p,1,3,1)
    return v.broadcast_to([128, L, 3, 1]).broadcast_to([128, L, 3, 4])


_NC_CACHE = None


def kernel(**inputs) -> np.ndarray:
    global _NC_CACHE
    shape = np.asarray(inputs["shape"], np.float32)
    body_pose = np.asarray(inputs["body_pose"], np.float32)
    pelvis_rotation = np.asarray(inputs["pelvis_rotation"], np.float32)
    global_translation = np.asarray(inputs["global_translation"], np.float32)

    dirsA, dirsB, w4, rhsj = _prep_consts(
        np.asarray(inputs["v_template"], np.float32),
        np.asarray(inputs["shapedirs"], np.float32),
        np.asarray(inputs["posedirs"], np.float32),
        np.asarray(inputs["lbs_weights"], np.float32),
        np.asarray(inputs["J_regressor"], np.float32),
    )

    full_pose = np.concatenate(
        [pelvis_rotation[:, None, :], body_pose], axis=1
    ).reshape(B, J * 3)

    if _NC_CACHE is None:
        _NC_CACHE = build_nc()
    nc = _NC_CACHE

    in_maps = []
    for c in range(NCORES):
        sl = slice(c * BL, (c + 1) * BL)
        in_maps.append({
            "shp": np.ascontiguousarray(shape[sl]),
            "pose": np.ascontiguousarray(full_pose[sl]),
            "gt": np.ascontiguousarray(global_translation[sl]),
            "rhsj": rhsj,
            "dirsA": dirsA,
            "dirsB": dirsB,
            "w4": w4,
        })

    import os as _os
    trace = bool(_os.environ.get("KERNEL_TRACE"))
    res = bass_utils.run_bass_kernel_spmd(
        nc, in_maps, core_ids=list(range(NCORES)), trace=trace
    )
    if trace:
        print(f"HW exec time: {res.exec_time_ns} ns")
        print(f"mean exec time: {res.mean_exec_time_ns} ns")
        print(f"trace: {res.instructions_and_trace[1] if res.instructions_and_trace else None}")
        kernel.last_result = res
    outs = [res.results[c]["out"] for c in range(NCORES)]
    full = np.concatenate(outs, axis=0)              # (1024, 3*VP)
    return full.reshape(B, VP, 3)[:, :V, :].copy()


# revision 14
# speedup vs baseline: 1.1766x; 1.1766x over previous
"""SMPL forward (shape blendshapes + pose blendshapes + FK + LBS) on 8 TRN2 NeuronCores.

Data-parallel: batch 1024 -> 128 per core. Layout: batch on the 128 SBUF
partitions; vertex streams along the free dimension, y/x-planar, V padded
7168 = 14 tiles of 512.

Pipeline per core:
  stage A (f32, small): Rodrigues rotations, joints from shape (host-folded
    J_regressor algebra), forward kinematics over the joint tree, A_rel.
  stage B (bf16 matmuls + elementwise): v_posed via one accumulated matmul
    [beta,1,pose_feat] @ [shapedirs;v_template;posedirs]; per-vertex blended
    transforms Tv_c = A_rel_c^T @ lbs_weights^T (12 components, 4-way
    row-packed K=24 matmuls); apply verts = Tv[:, :3,:3] p + Tv[:, :3,3].
"""

import sys

for _p in ("/opt/trn_rl_repo",):
    if _p not in sys.path:
        sys.path.append(_p)

import numpy as np
import ml_dtypes

import concourse.bass as bass
import concourse.mybir as mybir
from concourse.tile import TileContext
from concourse.vector_clock import ScopedClock
from concourse.masks import make_identity
from concourse import bass_utils

BF16 = ml_dtypes.bfloat16

B, V, J, NB = 1024, 6890, 24, 10
NCORES = 8
BL = B // NCORES            # 128 batches per core
VP = 7168                   # padded vertex count (14 * 512)
VT = 512                    # vertex tile
NT = VP // VT               # 14
KP = NB + 1 + (J - 1) * 9   # 218 = [beta(10), 1, pose_feat(207)]

PARENTS = np.array(
    [-1, 0, 0, 0, 1, 2, 3, 4, 5, 6, 7, 8, 9, 9, 9, 12, 13, 14, 16, 17, 18, 19, 20, 21],
    dtype=np.int32,
)

# FK levels: (joint_start, count, parent_start, parent_step) — parents are
# either contiguous (step 1 in joint index) or a single repeated joint (step 0).
FK_LEVELS = [
    (1, 3, 0, 0),
    (4, 3, 1, 1),
    (7, 3, 4, 1),
    (10, 3, 7, 1),
    (13, 2, 9, 0),
    (15, 3, 12, 1),
    (18, 2, 16, 1),
    (20, 2, 18, 1),
    (22, 2, 20, 1),
]

F32 = mybir.dt.float32
BF = mybir.dt.bfloat16


def _patched_drain_and_barrier(self, tick_clock, wait_clock):
    # This container's walrus accepts at most ONE sync-wait on a CTRL/Drain
    # instruction. Spread the end-of-kernel waits over SP NOPs, one each.
    nc = self.nc
    placeholders = [nc.sync.nop(nofuse=True) for _ in range(31)]
    drain_inst = nc.sync.drain()
    wait_clock.add_sem_waits(
        drain_inst.ins, ScopedClock({None: tick_clock.global_clock})
    )
    si = drain_inst.ins.sync_info
    waits = list(si.on_wait) if si and si.on_wait else []
    if len(waits) > 1:
        assert len(waits) - 1 <= len(placeholders), f"{len(waits)} drain waits"
        for ph, w in zip(placeholders, waits[:-1]):
            ph.ins.sync_info = mybir.SyncInfo(on_wait=[w], on_update=[])
        upd = list(si.on_update) if si.on_update else []
        drain_inst.ins.sync_info = mybir.SyncInfo(on_wait=[waits[-1]], on_update=upd)

    nc.all_engine_barrier()
    assert self.sems is not None
    popped = nc._tile_sem_poison_stack.pop()
    assert popped is self._sem_poison
    nc.clear_and_free_semaphores(list(self.sems.allocated().values()))
    nc.all_engine_barrier()


TileContext._drain_and_barrier = _patched_drain_and_barrier


def _split_multi_waits(nc):
    """This container's walrus accepts at most one sync-wait per instruction.
    Move surplus waits onto same-engine NOPs inserted just before."""
    uid = 0
    for fn in nc.m.functions:
        for bb in fn.blocks:
            changed = False
            new = []
            for inst in bb.instructions:
                si = inst.sync_info
                waits = list(si.on_wait) if si and si.on_wait else []
                if len(waits) > 1:
                    changed = True
                    for w in waits[:-1]:
                        uid += 1
                        nop = mybir.InstNoOp(
                            name=f"I-waitsplit-{uid}", ins=[], outs=[]
                        )
                        nop.engine = inst.engine
                        nop.sync_info = mybir.SyncInfo(on_wait=[w], on_update=[])
                        new.append(nop)
                    upd = list(si.on_update) if si.on_update else []
                    inst.sync_info = mybir.SyncInfo(
                        on_wait=[waits[-1]], on_update=upd
                    )
                new.append(inst)
            if changed:
                bb.instructions = new
    return nc


# --------------------------------------------------------------------------
# Host-side constant folding (model buffers only — no batch inputs touched)
# --------------------------------------------------------------------------

def _prep_consts(v_template, shapedirs, posedirs, lbs_weights, J_regressor):
    # dirs (218, 3*VP), y-planar columns: col = y*VP + v.
    # Row NB (the v_template row) is left at ZERO: the template term is
    # folded into the PE "base" matmul via c0w instead, so the matmul with
    # dirs yields only the correction delta = S beta + P theta.
    dirs = np.zeros((KP, 3 * VP), np.float32)
    dirs[0:NB].reshape(NB, 3, VP)[:, :, :V] = shapedirs.transpose(2, 1, 0)
    dirs[NB + 1:].reshape(207, 3, VP)[:, :, :V] = (
        posedirs.reshape(207, V, 3).transpose(0, 2, 1)
    )
    dirsA = np.ascontiguousarray(dirs[:128]).astype(BF16)
    dirsB = np.ascontiguousarray(dirs[128:]).astype(BF16)

    # lbs weights transposed, replicated on partition strips 0/32/64
    # (strip 96 unused by the Tv rounds)
    w4 = np.zeros((128, VP), np.float32)
    for i in range(3):
        w4[32 * i:32 * i + J, :V] = lbs_weights.T
    w4 = w4.astype(BF16)

    # base-term rhs (128, VP): rows 32y+j = w[v,j]*vt[v,y]; rows 96+j = w[v,j]
    # so that  ART_x.T @ c0w  =  (R~ vt)_x + t~_x  directly.
    c0w = np.zeros((128, VP), np.float32)
    for y in range(3):
        c0w[32 * y:32 * y + J, :V] = lbs_weights.T * v_template.T[y][None, :]
    c0w[96:96 + J, :V] = lbs_weights.T
    c0w = c0w.astype(BF16)

    # joints = J_regressor @ v_shaped folded to:  joints = base + jd @ beta
    jd = np.einsum("jv,vcl->jcl", J_regressor, shapedirs)  # (24,3,10)
    bj = J_regressor @ v_template                          # (24,3)
    abs_blk = np.concatenate([jd.reshape(J * 3, NB).T, bj.reshape(1, J * 3)], 0)
    # rel block: rel[j] = abs[j] - abs[parent[j]]
    rel = abs_blk.reshape(NB + 1, J, 3).copy()
    rel[:, 1:, :] -= abs_blk.reshape(NB + 1, J, 3)[:, PARENTS[1:], :]
    rhsj = np.concatenate([rel.reshape(NB + 1, J * 3), abs_blk], axis=1)  # (11,144)
    return dirsA, dirsB, w4, c0w, rhsj.astype(np.float32)


# --------------------------------------------------------------------------
# Device kernel
# --------------------------------------------------------------------------

def build_nc():
    nc = bass.Bass()

    shp = nc.dram_tensor("shp", [BL, NB], F32, kind="ExternalInput")
    pose = nc.dram_tensor("pose", [BL, J * 3], F32, kind="ExternalInput")
    gt = nc.dram_tensor("gt", [BL, 3], F32, kind="ExternalInput")
    rhsj = nc.dram_tensor("rhsj", [NB + 1, 144], F32, kind="ExternalInput")
    dirsA = nc.dram_tensor("dirsA", [128, 3 * VP], BF, kind="ExternalInput")
    dirsB = nc.dram_tensor("dirsB", [KP - 128, 3 * VP], BF, kind="ExternalInput")
    w4 = nc.dram_tensor("w4", [128, VP], BF, kind="ExternalInput")
    c0w = nc.dram_tensor("c0w", [128, VP], BF, kind="ExternalInput")
    out = nc.dram_tensor("out", [BL, 3 * VP], F32, kind="ExternalOutput")

    add = mybir.AluOpType.add

    with TileContext(nc) as tc:
        with (
            tc.tile_pool(name="const", bufs=1) as constp,
            tc.tile_pool(name="stagea", bufs=1) as sa,
            tc.tile_pool(name="work", bufs=2) as wk,
            tc.tile_pool(name="tvpool", bufs=2) as tvp,
            tc.tile_pool(name="ppool", bufs=2) as ppl,
            tc.tile_pool(name="outp", bufs=2) as outp,
        ):
            # ---- resident constants ----
            DA = constp.tile([128, 3 * VP], BF)
            DB = constp.tile([KP - 128, 3 * VP], BF)
            W4 = constp.tile([128, VP], BF)
            C0W = constp.tile([128, VP], BF)
            IDENT = constp.tile([128, 128], F32)
            nc.sync.dma_start(DA[:], dirsA[:])
            nc.sync.dma_start(DB[:], dirsB[:])
            nc.sync.dma_start(W4[:], w4[:])
            nc.sync.dma_start(C0W[:], c0w[:])
            make_identity(nc, IDENT[:])

            # ---- stage A inputs ----
            POSE = sa.tile([128, J * 3], F32)
            PB = sa.tile([128, KP], F32)          # [beta | 1 | pose_feat]
            G = sa.tile([128, 3], F32)
            RJ = sa.tile([NB + 1, 144], F32)
            nc.sync.dma_start(POSE[:], pose[:])
            nc.sync.dma_start(PB[:, 0:NB], shp[:])
            nc.sync.dma_start(G[:], gt[:])
            nc.sync.dma_start(RJ[:], rhsj[:])
            nc.vector.memset(PB[:, NB:NB + 1], 1.0)

            # ---- Rodrigues ----
            SQ = sa.tile([128, J * 3], F32)
            TH = sa.tile([128, J], F32)
            SN = sa.tile([128, J], F32)
            CS = sa.tile([128, J], F32)
            OMC = sa.tile([128, J], F32)
            IT = sa.tile([128, J], F32)
            AX = sa.tile([128, J * 3], F32)
            SAT = sa.tile([128, J * 3], F32)
            R = sa.tile([128, J * 9], F32)

            EPS = sa.tile([128, 1], F32)
            HPI = sa.tile([128, 1], F32)
            nc.vector.memset(EPS[:], 1e-8)
            nc.vector.memset(HPI[:], float(np.pi / 2))

            nc.vector.tensor_mul(SQ[:], POSE[:], POSE[:])
            nc.vector.tensor_reduce(
                TH[:], SQ[:].rearrange("p (j y) -> p j y", y=3),
                axis=mybir.AxisListType.X, op=add,
            )
            nc.scalar.activation(TH[:], TH[:], mybir.ActivationFunctionType.Sqrt,
                                 bias=EPS[:])
            nc.scalar.activation(SN[:], TH[:], mybir.ActivationFunctionType.Sin)
            nc.scalar.activation(CS[:], TH[:], mybir.ActivationFunctionType.Sin,
                                 bias=HPI[:])
            nc.vector.reciprocal(IT[:], TH[:])

            p_j3 = lambda t: t[:].rearrange("p (j y) -> p j y", y=3)
            p_j1 = lambda t: t[:].rearrange("p (j o) -> p j o", o=1)
            nc.vector.tensor_mul(
                p_j3(AX), p_j3(POSE), p_j1(IT).broadcast_to([128, J, 3])
            )
            R4 = R[:].rearrange("p (j x y) -> p j x y", x=3, y=3)
            nc.vector.tensor_mul(
                R4,
                p_j3(AX).unsqueeze(3).broadcast_to([128, J, 3, 3]),
                p_j3(AX).unsqueeze(2).broadcast_to([128, J, 3, 3]),
            )
            # (1 - cos)
            nc.scalar.activation(OMC[:], CS[:], mybir.ActivationFunctionType.Copy,
                                 bias=1.0, scale=-1.0)
            nc.vector.tensor_mul(
                R4, R4, p_j1(OMC).unsqueeze(3).broadcast_to([128, J, 3, 3])
            )
            R9 = R[:].rearrange("p (j d) -> p j d", d=9)
            Rdiag = R9[:, :, 0:9:4]
            nc.vector.tensor_add(
                Rdiag, Rdiag, p_j1(CS).broadcast_to([128, J, 3])
            )
            nc.vector.tensor_mul(
                p_j3(SAT), p_j3(AX), p_j1(SN).broadcast_to([128, J, 3])
            )
            SA3 = SAT[:].rearrange("p (j a) -> p j a", a=3)
            # +s*K off-diagonals: (dst_in_9, axis_comp, sign)
            for off, a, sgn in ((1, 2, -1), (2, 1, +1), (3, 2, +1),
                                (5, 0, -1), (6, 1, -1), (7, 0, +1)):
                fn = nc.vector.tensor_add if sgn > 0 else nc.vector.tensor_sub
                fn(R9[:, :, off:off + 1], R9[:, :, off:off + 1], SA3[:, :, a:a + 1])

            # ---- pose_feat into PB ----
            nc.vector.tensor_copy(PB[:, NB + 1:], R[:, 9:])
            PBd = PB[:, NB + 1:].rearrange("p (j d) -> p j d", d=9)[:, :, 0:9:4]
            nc.vector.tensor_scalar_add(PBd, PBd, -1.0)

            # ---- transpose PB -> lhsT chunks; joints matmul ----
            psA_cm = tc.tile_pool(name="psA", bufs=2, space="PSUM")
            psA = psA_cm.__enter__()
            PBT1 = sa.tile([128, 128], BF)
            PBT2 = sa.tile([KP - 128, 128], BF)
            JT = sa.tile([NB + 1, 128], F32)
            t1 = psA.tile([128, 128], F32, tag="ta")
            nc.tensor.transpose(t1[:], PB[:, 0:128], IDENT[:])
            nc.vector.tensor_copy(PBT1[:], t1[:])
            nc.vector.tensor_copy(JT[:], t1[0:NB + 1, :])
            t2 = psA.tile([KP - 128, 128], F32, tag="ta")
            nc.tensor.transpose(t2[:], PB[:, 128:KP], IDENT[:])
            nc.vector.tensor_copy(PBT2[:], t2[:])

            JNT = sa.tile([128, 144], F32)
            jp = psA.tile([128, 144], F32, tag="ta")
            nc.tensor.matmul(jp[:], JT[:], RJ[:], start=True, stop=True)
            nc.vector.tensor_copy(JNT[:], jp[:])

            # ---- FK ----
            TL = sa.tile([128, J * 12], F32)
            TW = sa.tile([128, J * 12], F32)
            TMP = sa.tile([128, 36], F32)
            TLv = TL[:].rearrange("p (j r c) -> p j r c", r=3, c=4)
            TWv = TW[:].rearrange("p (j r c) -> p j r c", r=3, c=4)
            RELv = JNT[:, 0:72].rearrange("p (j k) -> p j k", k=3)
            ABSv = JNT[:, 72:144].rearrange("p (j k) -> p j k", k=3)
            nc.vector.tensor_copy(
                TLv[:, :, :, 0:3], R[:].rearrange("p (j r y) -> p j r y", r=3, y=3)
            )
            nc.vector.tensor_copy(TLv[:, :, :, 3:4], RELv.unsqueeze(3))
            nc.vector.tensor_copy(TWv[:, 0:1, :, :], TLv[:, 0:1, :, :])

            TMPv = TMP[:].rearrange("p (j r c) -> p j r c", r=3, c=4)
            for j0, L, pstart, pstep in FK_LEVELS:
                outv = TWv[:, j0:j0 + L, :, :]
                tmpv = TMPv[:, 0:L, :, :]
                if pstep == 0:
                    parv = lambda k: _bc2(TWv, pstart, L, k)
                else:
                    parv = lambda k: TWv[:, pstart:pstart + L, :, k:k + 1] \
                        .broadcast_to([128, L, 3, 4])
                locv = lambda k: TLv[:, j0:j0 + L, k:k + 1, :] \
                    .broadcast_to([128, L, 3, 4])
                nc.vector.tensor_mul(outv, parv(0), locv(0))
                nc.vector.tensor_mul(tmpv, parv(1), locv(1))
                nc.vector.tensor_add(outv, outv, tmpv)
                nc.vector.tensor_mul(tmpv, parv(2), locv(2))
                nc.vector.tensor_add(outv, outv, tmpv)
                if pstep == 0:
                    ptr = TWv[:, pstart:pstart + 1, :, 3:4].broadcast_to([128, L, 3, 1])
                else:
                    ptr = TWv[:, pstart:pstart + L, :, 3:4]
                nc.vector.tensor_add(
                    TWv[:, j0:j0 + L, :, 3:4], TWv[:, j0:j0 + L, :, 3:4], ptr
                )

            # ---- A_rel translation: tcr = tw - Rw @ j_abs + g ----
            M = sa.tile([128, J * 9], F32)
            TC = sa.tile([128, J * 3], F32)
            TCR = sa.tile([128, J * 3], F32)
            Mv = M[:].rearrange("p (j r k) -> p j r k", r=3, k=3)
            nc.vector.tensor_mul(
                Mv, TWv[:, :, :, 0:3],
                ABSv.unsqueeze(2).broadcast_to([128, J, 3, 3]),
            )
            TCv = TC[:].rearrange("p (j r) -> p j r", r=3)
            nc.vector.tensor_reduce(TCv, Mv, axis=mybir.AxisListType.X, op=add)
            TCRv = TCR[:].rearrange("p (j r) -> p j r", r=3)
            nc.vector.tensor_sub(
                TCRv.unsqueeze(3), TWv[:, :, :, 3:4], TCv.unsqueeze(3)
            )
            nc.vector.tensor_add(
                TCRv.unsqueeze(3), TCRv.unsqueeze(3),
                G[:].rearrange("p (o c) -> p o c", o=1).unsqueeze(3)
                .broadcast_to([128, J, 3, 1]),
            )

            # ---- assemble AR (c-major, 32-padded j) and transpose ----
            AR = sa.tile([128, 384], F32)
            nc.vector.memset(AR[:], 0.0)
            ARv = AR[:].rearrange("p (r co j) -> p r co j", r=3, co=4, j=32)
            nc.vector.tensor_copy(
                ARv[:, :, 0:3, 0:J],
                TWv[:, :, :, 0:3].transpose([0, 2, 3, 1]),
            )
            nc.vector.tensor_copy(
                ARv[:, :, 3:4, 0:J],
                TCRv.transpose([0, 2, 1]).unsqueeze(2),
            )
            ART = []
            for i in range(3):
                ta = psA.tile([128, 128], F32, tag="ta")
                nc.tensor.transpose(ta[:], AR[:, 128 * i:128 * (i + 1)], IDENT[:])
                asb = sa.tile([128, 128], BF, tag=f"art{i}")
                nc.vector.tensor_copy(asb[:], ta[:])
                ART.append(asb)

            psA_cm.__exit__(None, None, None)

            # ---- stage B: per-vertex-tile stream ----
            # PSUM budget (8 banks): delta 2 + base 3 + Tv 3 = 8
            psP_cm = tc.tile_pool(name="psP", bufs=2, space="PSUM")
            psB_cm = tc.tile_pool(name="psB", bufs=3, space="PSUM")
            psT_cm = tc.tile_pool(name="psT", bufs=3, space="PSUM")
            psP = psP_cm.__enter__()
            psB = psB_cm.__enter__()
            psT = psT_cm.__enter__()
            for t in range(NT):
                # delta = S beta + P theta  (template row of dirs is zero)
                P = []
                for y in range(3):
                    pp = psP.tile([128, VT], F32, tag="pp")
                    c0 = y * VP + t * VT
                    nc.tensor.matmul(pp[:], PBT1[:], DA[:, c0:c0 + VT],
                                     start=True, stop=False)
                    nc.tensor.matmul(pp[:], PBT2[:], DB[:, c0:c0 + VT],
                                     start=False, stop=True)
                    psb = ppl.tile([128, VT], BF, tag=f"p{y}")
                    nc.scalar.copy(psb[:], pp[:])
                    P.append(psb)

                # base_x = (R~ vt)_x + t~_x  via K=128 matmul against c0w
                BASE = []
                for x in range(3):
                    bs = psB.tile([128, VT], F32, tag="bs")
                    nc.tensor.matmul(
                        bs[:], ART[x][:], C0W[:, t * VT:(t + 1) * VT],
                        start=True, stop=True,
                    )
                    BASE.append(bs)

                # R~ components: per x-round, 3 row-packed K=24 matmuls (y strips)
                TV = []
                for x in range(3):
                    tvps = []
                    for y in range(3):
                        tv = psT.tile([128, VT], F32, tag="tv")
                        nc.tensor.matmul(
                            tv[:],
                            ART[x][32 * y:32 * y + J, :],
                            W4[32 * y:32 * y + J, t * VT:(t + 1) * VT],
                            start=True, stop=True,
                            tile_position=(32 * y, 0),
                        )
                        tvps.append(tv)
                    for y in range(3):
                        tsb = tvp.tile([128, VT], BF, tag=f"tv{3 * x + y}")
                        nc.scalar.copy(tsb[:], tvps[y][:])
                        TV.append(tsb)

                # apply: out_x = (T_x0 d0 + T_x1 d1 + T_x2 d2) + base_x
                OUT = outp.tile([128, 3 * VT], F32, tag="out")
                OUTv = OUT[:].rearrange("p (v x) -> p v x", x=3)
                for x in range(3):
                    m0 = wk.tile([128, VT], BF, tag="m0")
                    m1 = wk.tile([128, VT], BF, tag="m1")
                    m2 = wk.tile([128, VT], BF, tag="m2")
                    nc.vector.tensor_mul(m0[:], TV[3 * x + 0][:], P[0][:])
                    nc.vector.tensor_mul(m1[:], TV[3 * x + 1][:], P[1][:])
                    nc.vector.tensor_mul(m2[:], TV[3 * x + 2][:], P[2][:])
                    nc.vector.tensor_add(m0[:], m0[:], m1[:])
                    nc.vector.tensor_add(m0[:], m0[:], m2[:])
                    nc.vector.tensor_add(
                        OUTv[:, :, x:x + 1], m0[:].unsqueeze(2),
                        BASE[x][:].unsqueeze(2),
                    )
                nc.sync.dma_start(out[:, t * 3 * VT:(t + 1) * 3 * VT], OUT[:])
            psT_cm.__exit__(None, None, None)
            psB_cm.__exit__(None, None, None)
            psP_cm.__exit__(None, None, None)

    return _split_multi_waits(nc)


def _bc2(TWv, pstart, L, k):
    # broadcast a single parent's column k over L joints and 4 columns
    v = TWv[:, pstart:pstart + 1, :, k:k + 1]           # (p,1,3,1)
    return v.broadcast_to([128, L, 3, 1]).broadcast_to([128, L, 3, 4])


_NC_CACHE = None


def kernel(**inputs) -> np.ndarray:
    global _NC_CACHE
    shape = np.asarray(inputs["shape"], np.float32)
    body_pose = np.asarray(inputs["body_pose"], np.float32)
    pelvis_rotation = np.asarray(inputs["pelvis_rotation"], np.float32)
    global_translation = np.asarray(inputs["global_translation"], np.float32)

    dirsA, dirsB, w4, c0w, rhsj = _prep_consts(
        np.asarray(inputs["v_template"], np.float32),
        np.asarray(inputs["shapedirs"], np.float32),
        np.asarray(inputs["posedirs"], np.float32),
        np.asarray(inputs["lbs_weights"], np.float32),
        np.asarray(inputs["J_regressor"], np.float32),
    )

    full_pose = np.concatenate(
        [pelvis_rotation[:, None, :], body_pose], axis=1
    ).reshape(B, J * 3)

    if _NC_CACHE is None:
        _NC_CACHE = build_nc()
    nc = _NC_CACHE

    in_maps = []
    for c in range(NCORES):
        sl = slice(c * BL, (c + 1) * BL)
        in_maps.append({
            "shp": np.ascontiguousarray(shape[sl]),
            "pose": np.ascontiguousarray(full_pose[sl]),
            "gt": np.ascontiguousarray(global_translation[sl]),
            "rhsj": rhsj,
            "dirsA": dirsA,
            "dirsB": dirsB,
            "w4": w4,
            "c0w": c0w,
        })

    import os as _os
    trace = bool(_os.environ.get("KERNEL_TRACE"))
    res = bass_utils.run_bass_kernel_spmd(
        nc, in_maps, core_ids=list(range(NCORES)), trace=trace
    )
    if trace:
        print(f"HW exec time: {res.exec_time_ns} ns")
        print(f"mean exec time: {res.mean_exec_time_ns} ns")
        print(f"trace: {res.instructions_and_trace[1] if res.instructions_and_trace else None}")
        kernel.last_result = res
    outs = [res.results[c]["out"] for c in range(NCORES)]
    full = np.concatenate(outs, axis=0)              # (1024, 3*VP)
    return full.reshape(B, VP, 3)[:, :V, :].copy()
